# revision 1
# baseline (speedup 1.0000x reference)
"""Trainium2 Bass kernel for the Nawrot downsampler-upsampler module.

Per-core (data-parallel over batch, 1 example per NeuronCore):
  1. PE prefix-sums of x along L (triangular matmul + rank-1 carry) -> P table in DRAM
  2. PE-transpose x -> fp32 MLP (relu(x@W1+b1)) -> logits via ones-matmul partition reduce
  3. boundary bits from logits + logistic noise; cumsum via tensor_tensor_scan
  4. scatter boundary positions into a p-table (indirect DMA, OOB-drop for non-boundaries)
  5. gather P rows at segment endpoints, subtract, scale by 1/count -> short table
  6. final indirect row-gather by seg_inc -> output
"""
import sys

sys.path.insert(0, "/opt/trn_rl_repo")

import numpy as np
from contextlib import ExitStack

import concourse.bass as bass
import concourse.bacc as bacc
import concourse.tile as tile
from concourse import mybir
from concourse.masks import make_identity

F32 = mybir.dt.float32
I32 = mybir.dt.int32
OP = mybir.AluOpType
ACT = mybir.ActivationFunctionType

B = 8
L_FULL = 2048
D_FULL = 1024
N_CORES = 8


def build(L=L_FULL, D=D_FULL):
    P = 128
    NLT = L // P          # l-tiles
    ND = D // P           # d-tiles
    CPT = L // P          # scan columns per partition (l = p*CPT + c)
    DC = min(512, D)      # matmul free-dim chunk
    NDC = D // DC
    LCH = min(512, L)     # l-chunk for MLP matmuls
    NLC = L // LCH

    nc = bacc.Bacc("TRN2", target_bir_lowering=False, debug=False, num_devices=N_CORES)

    x_d = nc.dram_tensor("x", [L, D], F32, kind="ExternalInput").ap()
    noise_d = nc.dram_tensor("noise", [L], F32, kind="ExternalInput").ap()
    w1_d = nc.dram_tensor("W1", [D, D], F32, kind="ExternalInput").ap()
    b1_d = nc.dram_tensor("b1", [D], F32, kind="ExternalInput").ap()
    w2_d = nc.dram_tensor("W2", [D], F32, kind="ExternalInput").ap()
    b2_d = nc.dram_tensor("b2", [1], F32, kind="ExternalInput").ap()
    null_d = nc.dram_tensor("null_group", [1, 1, D], F32, kind="ExternalInput").ap()
    up_d = nc.dram_tensor("up", [L, D], F32, kind="ExternalOutput").ap()

    with tile.TileContext(nc) as tc, ExitStack() as ctx:
        const = ctx.enter_context(tc.tile_pool(name="const", bufs=1))
        dram = ctx.enter_context(tc.tile_pool(name="dram", bufs=1, space="DRAM"))
        xpool = ctx.enter_context(tc.tile_pool(name="xp", bufs=3))
        xtp = ctx.enter_context(tc.tile_pool(name="xtp", bufs=2))
        stage = ctx.enter_context(tc.tile_pool(name="stage", bufs=3))
        small = ctx.enter_context(tc.tile_pool(name="small", bufs=1))
        crowp = ctx.enter_context(tc.tile_pool(name="crowp", bufs=2))
        gpool = ctx.enter_context(tc.tile_pool(name="gp", bufs=2))
        logp = ctx.enter_context(tc.tile_pool(name="logp", bufs=2))
        psA = ctx.enter_context(tc.tile_pool(name="psA", bufs=2, space="PSUM"))
        psT = ctx.enter_context(tc.tile_pool(name="psT", bufs=2, space="PSUM"))
        psM = ctx.enter_context(tc.tile_pool(name="psM", bufs=2, space="PSUM"))

        # ---------------- DRAM scratch ----------------
        P_scr = dram.tile([L + 2, D], F32)   # prefix sums; row L = zeros, row L+1 = null
        lg_scr = dram.tile([L, 1], F32)      # logits row bounce
        pe_scr = dram.tile([L, 1], I32)      # per-token segment-end row index
        ps_scr = dram.tile([L, 1], I32)      # per-token segment-start-minus-one row index
        r_scr = dram.tile([L, 1], F32)       # per-token reciprocal count

        # ---------------- constants ----------------
        const_dmas = []
        w1_sb = const.tile([P, ND, D], F32)
        const_dmas.append(nc.sync.dma_start(out=w1_sb[:], in_=w1_d.rearrange("(i p) n -> p i n", p=P)))
        b1_sb = const.tile([P, ND], F32)
        const_dmas.append(nc.sync.dma_start(out=b1_sb[:], in_=b1_d.rearrange("(o p) -> p o", p=P)))
        w2_sb = const.tile([P, ND], F32)
        const_dmas.append(nc.sync.dma_start(out=w2_sb[:], in_=w2_d.rearrange("(o p) -> p o", p=P)))
        b2_sb = const.tile([1, 1], F32)
        const_dmas.append(nc.sync.dma_start(out=b2_sb[:], in_=b2_d.rearrange("(a b) -> a b", a=1)))
        null_sb = const.tile([1, D], F32)
        const_dmas.append(nc.sync.dma_start(out=null_sb[:], in_=null_d[0, 0, :].rearrange("(a d) -> a d", a=1)))

        ident = const.tile([P, P], F32)
        make_identity(nc, ident[:])

        pio = const.tile([P, 1], F32)
        nc.gpsimd.iota(pio[:], pattern=[[0, 1]], base=0, channel_multiplier=1,
                       allow_small_or_imprecise_dtypes=True)
        fio = const.tile([P, P], F32)
        nc.gpsimd.iota(fio[:], pattern=[[1, P]], base=0, channel_multiplier=0,
                       allow_small_or_imprecise_dtypes=True)
        # ut[k, m] = 1 if k <= m   (inclusive prefix lhsT)
        ut = const.tile([P, P], F32)
        nc.vector.tensor_scalar(out=ut[:], in0=fio[:], scalar1=pio[:], scalar2=None, op0=OP.is_ge)
        ones_row1 = const.tile([1, P], F32)
        nc.vector.memset(ones_row1[:], 1.0)
        ones_col = const.tile([P, 1], F32)
        nc.vector.memset(ones_col[:], 1.0)
        ones_1x1 = const.tile([1, 1], F32)
        nc.vector.memset(ones_1x1[:], 1.0)
        zero_row = const.tile([1, D], F32)
        nc.vector.memset(zero_row[:], 0.0)
        zeros_cpt = const.tile([P, CPT], F32)
        nc.vector.memset(zeros_cpt[:], 0.0)
        zrow128 = const.tile([1, P], F32)
        nc.vector.memset(zrow128[:], 0.0)
        iotp1 = const.tile([P, CPT], F32)   # l + 1 (l = p*CPT + c), exact in f32
        nc.gpsimd.iota(iotp1[:], pattern=[[1, CPT]], base=1, channel_multiplier=CPT,
                       allow_small_or_imprecise_dtypes=True)

        # shared bounds-check register (indirect DMAs otherwise allocate a
        # fresh gpsimd register each and exhaust the file)
        bc_gather = nc.gpsimd.to_reg(L + 1)

        # zeros row L and null row L+1 of the P table
        const_dmas.append(nc.sync.dma_start(out=P_scr[L:L + 1, :], in_=zero_row[:]))
        const_dmas.append(nc.sync.dma_start(out=P_scr[L + 1:L + 2, :], in_=null_sb[:]))

        # Collapse the fan of constant-load DMA lanes into one tick so later
        # matmuls don't exceed the per-instruction sync-wait slot limit.  The
        # barrier NOP itself is subject to the same limit, so first absorb the
        # DMA-lane semaphores into the SP clock with nops of <=4 deps each.
        from concourse.tile_rust import add_dep_helper as _adh
        for g in range(0, len(const_dmas), 4):
            spn = nc.sync.nop()
            for d in const_dmas[g:g + 4]:
                _adh(spn.ins, d.ins, sync=True, reason="const-lane coalesce")
        tc.strict_bb_all_engine_barrier()

        # ------- phases 1+2: per 512-token chunk: load, transpose, prefix, MLP -------
        crow_prev = None
        for lc in range(NLC):
            lsl = slice(lc * LCH, (lc + 1) * LCH)
            xT_ch = xtp.tile([P, ND, LCH], F32, tag="xT")  # xT[p, j, l_local]

            for ii in range(LCH // P):
                i = lc * (LCH // P) + ii
                x_t = xpool.tile([P, D], F32, tag="x")
                nc.sync.dma_start(out=x_t[:], in_=x_d[i * P:(i + 1) * P, :])

                # transposes in groups of 4 per PSUM bank
                for jg in range((ND + 3) // 4):
                    n_in_g = min(4, ND - jg * 4)
                    ps_t = psT.tile([P, 512], F32, tag="tr")
                    for jj in range(n_in_g):
                        j = jg * 4 + jj
                        nc.tensor.transpose(
                            out=ps_t[:, jj * P:(jj + 1) * P],
                            in_=x_t[:, j * P:(j + 1) * P],
                            identity=ident[:],
                        )
                    nc.vector.tensor_copy(
                        out=xT_ch[:, jg * 4:jg * 4 + n_in_g, ii * P:(ii + 1) * P],
                        in_=ps_t[:, :n_in_g * P].rearrange("p (j q) -> p j q", q=P),
                    )

                # prefix within tile + carry row from previous tiles
                psP = psA.tile([P, D], F32, tag="P")
                for dc in range(NDC):
                    sl = slice(dc * DC, (dc + 1) * DC)
                    nc.tensor.matmul(
                        psP[:, sl], lhsT=ut[:], rhs=x_t[:, sl],
                        start=True, stop=(i == 0),
                    )
                    if i > 0:
                        nc.tensor.matmul(
                            psP[:, sl], lhsT=ones_row1[:], rhs=crow_prev[:, sl],
                            start=False, stop=True,
                        )
                p_st = stage.tile([P, D], F32, tag="Pst")
                nc.vector.tensor_copy(out=p_st[:], in_=psP[:])
                # carry row = last row of the tile prefix; engines can't read a
                # single partition at offset 127, but DMA can
                crow = crowp.tile([1, D], F32, tag="crow")
                nc.sync.dma_start(out=crow[:], in_=p_st[P - 1:P, :])
                crow_prev = crow
                nc.sync.dma_start(out=P_scr[i * P:(i + 1) * P, :], in_=p_st[:])

            # MLP for this l-chunk
            logacc = logp.tile([P, LCH], F32, tag="logacc")
            for o in range(ND):
                psm = psM.tile([P, LCH], F32, tag="mlp")
                for i_ in range(ND):
                    nc.tensor.matmul(
                        psm[:],
                        lhsT=w1_sb[:, i_, o * P:(o + 1) * P],
                        rhs=xT_ch[:, i_, :],
                        start=(i_ == 0), stop=(i_ == ND - 1),
                    )
                hT = stage.tile([P, LCH], F32, tag="hT")
                nc.scalar.activation(
                    out=hT[:], in_=psm[:], func=ACT.Relu,
                    bias=b1_sb[:, o:o + 1], scale=1.0,
                )
                if o == 0:
                    nc.vector.tensor_scalar(
                        out=logacc[:], in0=hT[:],
                        scalar1=w2_sb[:, o:o + 1], scalar2=None, op0=OP.mult,
                    )
                else:
                    nc.vector.scalar_tensor_tensor(
                        out=logacc[:], in0=hT[:], scalar=w2_sb[:, o:o + 1],
                        in1=logacc[:], op0=OP.mult, op1=OP.add,
                    )

            # logits partial for this chunk: partition-reduce + bias, to DRAM
            pslg = psM.tile([1, LCH], F32, tag="mlp")
            nc.tensor.matmul(pslg[:], lhsT=ones_col[:], rhs=logacc[:], start=True, stop=True)
            lg_ch = stage.tile([1, LCH], F32, tag="lgch")
            nc.scalar.activation(
                out=lg_ch[:], in_=pslg[:], func=ACT.Identity,
                bias=b2_sb[:, 0:1], scale=1.0,
            )
            nc.sync.dma_start(
                out=lg_scr[lsl, 0].rearrange("(a l) -> a l", a=1), in_=lg_ch[:]
            )

        # ---------------- phase 3: boundary bits, cumsum ----------------
        lg16 = small.tile([P, CPT], F32, tag="lg16")
        nc.sync.dma_start(out=lg16[:], in_=lg_scr[:, 0].rearrange("(p c) -> p c", c=CPT))

        nz16 = small.tile([P, CPT], F32, tag="nz")
        nc.sync.dma_start(out=nz16[:], in_=noise_d.rearrange("(p c) -> p c", c=CPT))

        lnu = small.tile([P, CPT], F32, tag="lnu")
        nc.scalar.activation(out=lnu[:], in_=nz16[:], func=ACT.Ln)
        om = small.tile([P, CPT], F32, tag="om")
        nc.vector.tensor_scalar(
            out=om[:], in0=nz16[:], scalar1=1.0, scalar2=-1.0,
            op0=OP.subtract, op1=OP.mult,
        )  # (u - 1) * -1 = 1 - u
        ln1m = small.tile([P, CPT], F32, tag="ln1m")
        nc.scalar.activation(out=ln1m[:], in_=om[:], func=ACT.Ln)
        tt = small.tile([P, CPT], F32, tag="tt")
        nc.vector.tensor_tensor(out=tt[:], in0=lnu[:], in1=ln1m[:], op=OP.subtract)
        nc.vector.tensor_tensor(out=tt[:], in0=tt[:], in1=lg16[:], op=OP.add)
        hard = small.tile([P, CPT], F32, tag="hard")
        nc.vector.tensor_scalar(out=hard[:], in0=tt[:], scalar1=0.0, scalar2=None, op0=OP.is_gt)

        # ---- prefix-max scans: lb_inc (last boundary <= l), scan2 (boundary before it)
        def cross_part_max_scan(inclusive, tagp):
            """Combine per-partition inclusive max-scans into a global scan.

            Returns a (P, CPT) tile where each row has been max-ed with the
            running max of all previous partitions' row-maxima.
            """
            # row maxima -> (1, P) via matmul with identity rhs
            ps_r = psT.tile([P, 512], F32, tag="tr")
            nc.tensor.matmul(
                ps_r[0:1, 0:P], lhsT=inclusive[:, CPT - 1:CPT], rhs=ident[:],
                start=True, stop=True,
            )
            rowT = small.tile([1, P], F32, tag=tagp + "_rowT")
            nc.vector.tensor_copy(out=rowT[:], in_=ps_r[0:1, 0:P])
            # inclusive scan along the (1, P) row, then shift right one (exclusive)
            sc = small.tile([1, P], F32, tag=tagp + "_sc")
            nc.vector.tensor_tensor_scan(
                out=sc[:], data0=rowT[:], data1=zrow128[:],
                initial=-1.0, op0=OP.max, op1=OP.add,
            )
            exc = small.tile([1, P], F32, tag=tagp + "_exc")
            nc.vector.memset(exc[0:1, 0:1], -1.0)
            nc.vector.tensor_copy(out=exc[0:1, 1:P], in_=sc[0:1, 0:P - 1])
            # back to (P, 1) via rank-1 matmul with ones (1,1)
            ps_b = psT.tile([P, 512], F32, tag="tr")
            nc.tensor.matmul(
                ps_b[:, 0:1], lhsT=exc[:], rhs=ones_1x1[:], start=True, stop=True,
            )
            offm = small.tile([P, 1], F32, tag=tagp + "_offm")
            nc.vector.tensor_copy(out=offm[:], in_=ps_b[:, 0:1])
            out_t = small.tile([P, CPT], F32, tag=tagp + "_out")
            nc.vector.tensor_scalar(
                out=out_t[:], in0=inclusive[:], scalar1=offm[:], scalar2=None, op0=OP.max,
            )
            return out_t, offm

        # mi = hard ? l : -1  == (l+1)*hard - 1
        mi = small.tile([P, CPT], F32, tag="mi")
        nc.vector.tensor_tensor(out=mi[:], in0=iotp1[:], in1=hard[:], op=OP.mult)
        nc.vector.tensor_scalar(out=mi[:], in0=mi[:], scalar1=-1.0, scalar2=None, op0=OP.add)
        s1l = small.tile([P, CPT], F32, tag="s1l")
        nc.vector.tensor_tensor_scan(
            out=s1l[:], data0=mi[:], data1=zeros_cpt[:],
            initial=-1.0, op0=OP.max, op1=OP.add,
        )
        lb_inc, offm1 = cross_part_max_scan(s1l, "s1")

        # lbm1[l] = lb_inc[l-1] (token shift; layout l = p*CPT + c).
        # Column 0 of partition p is lb_inc at the end of partition p-1,
        # which is exactly the exclusive cross-partition max offm1.
        lbm1 = small.tile([P, CPT], F32, tag="lbm1")
        nc.vector.tensor_copy(out=lbm1[:, 0:1], in_=offm1[:])
        nc.vector.tensor_copy(out=lbm1[:, 1:CPT], in_=lb_inc[:, 0:CPT - 1])
        # mi2 = hard ? lbm1 : -1 == (lbm1+1)*hard - 1
        mi2 = small.tile([P, CPT], F32, tag="mi2")
        nc.vector.tensor_scalar(out=mi2[:], in0=lbm1[:], scalar1=1.0, scalar2=None, op0=OP.add)
        nc.vector.tensor_tensor(out=mi2[:], in0=mi2[:], in1=hard[:], op=OP.mult)
        nc.vector.tensor_scalar(out=mi2[:], in0=mi2[:], scalar1=-1.0, scalar2=None, op0=OP.add)
        s2l = small.tile([P, CPT], F32, tag="s2l")
        nc.vector.tensor_tensor_scan(
            out=s2l[:], data0=mi2[:], data1=zeros_cpt[:],
            initial=-1.0, op0=OP.max, op1=OP.add,
        )
        pb, _ = cross_part_max_scan(s2l, "s2")

        # cnt = lb_inc - pb ;  r = 1/(cnt + 1e-9), forced to 1.0 for null tokens
        cnt = small.tile([P, CPT], F32, tag="cnt")
        nc.vector.tensor_tensor(out=cnt[:], in0=lb_inc[:], in1=pb[:], op=OP.subtract)
        nc.vector.tensor_scalar(out=cnt[:], in0=cnt[:], scalar1=1e-9, scalar2=None, op0=OP.add)
        r_tok = small.tile([P, CPT], F32, tag="r_tok")
        nc.vector.reciprocal(out=r_tok[:], in_=cnt[:])
        mask0 = small.tile([P, CPT], F32, tag="mask0")
        nc.vector.tensor_scalar(out=mask0[:], in0=lb_inc[:], scalar1=-0.5, scalar2=None, op0=OP.is_gt)
        # r_tok = (r_tok - 1)*mask0 + 1
        nc.vector.tensor_scalar(out=r_tok[:], in0=r_tok[:], scalar1=-1.0, scalar2=None, op0=OP.add)
        nc.vector.tensor_tensor(out=r_tok[:], in0=r_tok[:], in1=mask0[:], op=OP.mult)
        nc.vector.tensor_scalar(out=r_tok[:], in0=r_tok[:], scalar1=1.0, scalar2=None, op0=OP.add)
        # pe = mask0 ? lb_inc : L+1 (null row)   == (lb_inc - (L+1))*mask0 + (L+1)
        pe_t = small.tile([P, CPT], F32, tag="pe_t")
        nc.vector.tensor_scalar(out=pe_t[:], in0=lb_inc[:], scalar1=-float(L + 1), scalar2=None, op0=OP.add)
        nc.vector.tensor_tensor(out=pe_t[:], in0=pe_t[:], in1=mask0[:], op=OP.mult)
        nc.vector.tensor_scalar(out=pe_t[:], in0=pe_t[:], scalar1=float(L + 1), scalar2=None, op0=OP.add)
        # ps = pb >= 0 ? pb : L (zeros row)      == (pb - L)*mask2 + L
        mask2 = small.tile([P, CPT], F32, tag="mask2")
        nc.vector.tensor_scalar(out=mask2[:], in0=pb[:], scalar1=-0.5, scalar2=None, op0=OP.is_gt)
        ps_t2 = small.tile([P, CPT], F32, tag="ps_t2")
        nc.vector.tensor_scalar(out=ps_t2[:], in0=pb[:], scalar1=-float(L), scalar2=None, op0=OP.add)
        nc.vector.tensor_tensor(out=ps_t2[:], in0=ps_t2[:], in1=mask2[:], op=OP.mult)
        nc.vector.tensor_scalar(out=ps_t2[:], in0=ps_t2[:], scalar1=float(L), scalar2=None, op0=OP.add)

        # int32 offsets + layout bounce (p*CPT+c) -> (128t+p) chunked
        pei = small.tile([P, CPT], I32, tag="pei")
        nc.vector.tensor_copy(out=pei[:], in_=pe_t[:])
        psi = small.tile([P, CPT], I32, tag="psi")
        nc.vector.tensor_copy(out=psi[:], in_=ps_t2[:])
        nc.sync.dma_start(out=pe_scr[:, 0].rearrange("(p c) -> p c", c=CPT), in_=pei[:])
        nc.sync.dma_start(out=ps_scr[:, 0].rearrange("(p c) -> p c", c=CPT), in_=psi[:])
        nc.sync.dma_start(out=r_scr[:, 0].rearrange("(p c) -> p c", c=CPT), in_=r_tok[:])
        pe2 = small.tile([P, NLT], I32, tag="pe2")
        nc.sync.dma_start(out=pe2[:], in_=pe_scr[:, 0].rearrange("(t p) -> p t", p=P))
        ps2 = small.tile([P, NLT], I32, tag="ps2")
        nc.sync.dma_start(out=ps2[:], in_=ps_scr[:, 0].rearrange("(t p) -> p t", p=P))
        r2 = small.tile([P, NLT], F32, tag="r2")
        nc.sync.dma_start(out=r2[:], in_=r_scr[:, 0].rearrange("(t p) -> p t", p=P))

        # ---- final: per 128-token chunk, gather segment-end/start prefix rows ----
        for t in range(NLT):
            gpe = gpool.tile([P, D], F32, tag="ge")
            nc.gpsimd.indirect_dma_start(
                out=gpe[:], out_offset=None, in_=P_scr[:],
                in_offset=bass.IndirectOffsetOnAxis(ap=pe2[:, t:t + 1], axis=0),
                bounds_check=bc_gather, oob_is_err=False,
            )
            gps = gpool.tile([P, D], F32, tag="gs")
            nc.gpsimd.indirect_dma_start(
                out=gps[:], out_offset=None, in_=P_scr[:],
                in_offset=bass.IndirectOffsetOnAxis(ap=ps2[:, t:t + 1], axis=0),
                bounds_check=bc_gather, oob_is_err=False,
            )
            upt = gpool.tile([P, D], F32, tag="up")
            nc.vector.tensor_tensor(out=upt[:], in0=gpe[:], in1=gps[:], op=OP.subtract)
            nc.vector.tensor_scalar(
                out=upt[:], in0=upt[:], scalar1=r2[:, t:t + 1], scalar2=None, op0=OP.mult,
            )
            nc.sync.dma_start(out=up_d[t * P:(t + 1) * P, :], in_=upt[:])

    nc.compile()
    return nc


_nc_cache = {}


def _get_nc(L, D):
    key = (L, D)
    if key not in _nc_cache:
        _nc_cache[key] = build(L, D)
    return _nc_cache[key]


def make_in_maps(inputs, n_cores=N_CORES):
    x = np.ascontiguousarray(np.asarray(inputs["x"], dtype=np.float32))
    noise = np.ascontiguousarray(np.asarray(inputs["noise"], dtype=np.float32))
    shared = {
        "W1": np.ascontiguousarray(np.asarray(inputs["W1"], dtype=np.float32)),
        "b1": np.ascontiguousarray(np.asarray(inputs["b1"], dtype=np.float32)),
        "W2": np.ascontiguousarray(np.asarray(inputs["W2"], dtype=np.float32)),
        "b2": np.ascontiguousarray(np.asarray(inputs["b2"], dtype=np.float32)),
        "null_group": np.ascontiguousarray(np.asarray(inputs["null_group"], dtype=np.float32)),
    }
    return [dict(shared, x=x[c], noise=noise[c]) for c in range(n_cores)]


def kernel(**inputs):
    from concourse.bass_utils import run_bass_kernel_spmd

    x = np.asarray(inputs["x"])
    b, L, D = x.shape
    assert b == N_CORES
    nc = _get_nc(L, D)
    in_maps = make_in_maps(inputs)
    res = run_bass_kernel_spmd(nc, in_maps, core_ids=list(range(N_CORES)))
    out = np.stack([res.results[c]["up"] for c in range(N_CORES)], axis=0)
    return out.astype(np.float32)

